# revision 41
# baseline (speedup 1.0000x reference)
"""Distributed GAT (2-layer, heads=1) on 8 TRN2 NeuronCores — v2.

Nodes partitioned by dst across 8 cores (12500/core, padded 12544=98*128).
Per layer: each core computes a combined bf16 row table
[h(bf16) | a_src-logit(fp32 in 2 bf16 slots) | 1.0 | pad] = 256B/row,
AllGathers it, then processes its dst tiles. Edge gathering uses batched
gpsimd dma_gather (one SWDGE instruction per (8-tile group x int16 row
range) instead of one indirect DMA per 128-edge chunk). One-hot scatter
matrices are host-staged bf16 and drive both the per-dst PSUM-accumulated
scatter matmul (with softmax denominator as a folded 'ones' column) and a
fused scalar_tensor_tensor ad-select+row-reduce. Scores/exp are computed
group-wide; per-chunk weighting runs on the scalar engine via
activation(scale=ex). Self-loop chunks load sequentially from the local
table and scatter through a staged identity.
"""
import sys
sys.path.insert(0, '/opt/trn_rl_repo')
import numpy as np
import ml_dtypes

import concourse.bass as bass
import concourse.bacc as bacc
import concourse.tile as tile
from concourse import mybir
from concourse.bass_utils import run_bass_kernel_spmd
from concourse.library_config import mlp

N_CORES = 8
N = 100000
NPC = N // N_CORES          # 12500 nodes per core
NT = 98                     # dst tiles per core
NPAD = NT * 128             # 12544 padded nodes per core
NFULL = N_CORES * NPAD      # 100352 padded global nodes
RANGE = NFULL // 4          # 25088 rows per int16-addressable table range
F1, H, O = 256, 64, 20
EW = 128                    # bf16 slots per table row (256B)
NEG_SLOPE = 0.2
GT = 8                      # tiles per group
AF = mybir.ActivationFunctionType
ALU = mybir.AluOpType
BF = mybir.dt.bfloat16
F32 = mybir.dt.float32


def _prep_edges(edge_index):
    """Group real edges per core by (dst tile, src range); chunk counts are
    maxed over cores so one SPMD program fits all. Returns per-core staged
    arrays (idx16 wrapped layout, bf16 one-hot) + the global chunk layout."""
    src = np.asarray(edge_index[0], dtype=np.int64)
    dst = np.asarray(edge_index[1], dtype=np.int64)
    sp_all = (src // NPC) * NPAD + (src % NPC)      # padded-global src id

    per_core = []
    counts = np.zeros((N_CORES, NT, 4), dtype=np.int64)
    for c in range(N_CORES):
        m = (dst // NPC) == c
        sp, ld = sp_all[m], (dst[m] % NPC).astype(np.int64)
        t = ld // 128
        w = ld % 128
        r = sp // RANGE
        order = np.lexsort((sp, r, t))
        sp, t, w, r = sp[order], t[order], w[order], r[order]
        np.add.at(counts[c], (t, r), 1)
        per_core.append((sp, t, w, r))

    K = np.ceil(counts.max(axis=0) / 128).astype(np.int64)   # [NT, 4] chunks
    # global chunk layout: groups of GT tiles; within a group chunks are
    # ordered (range, tile, k) so each range is one contiguous gather span.
    groups = []
    col = 0
    for g0 in range(0, NT, GT):
        tiles = list(range(g0, min(g0 + GT, NT)))
        spans = []          # per range: (start_col, n_chunks)
        tile_cols = {t: [] for t in tiles}
        start_col = col
        for r in range(4):
            s = col
            for t in tiles:
                for k in range(K[t][r]):
                    tile_cols[t].append(col)
                    col += 1
            spans.append((s, col - s))
        groups.append(dict(tiles=tiles, spans=spans, tile_cols=tile_cols,
                           start=start_col, n=col - start_col))
    totch = col

    staged = []
    for c in range(N_CORES):
        sp, t, w, r = per_core[c]
        # chunk-slot assignment: within (t, r), edges fill chunks in order
        k_off = np.zeros((NT, 4), dtype=np.int64)
        k_off[:, 1:] = np.cumsum(K, axis=1)[:, :-1]
        base = np.zeros((NT, 4), dtype=np.int64)    # first col of (t,r)
        pos_in = np.zeros(len(sp), dtype=np.int64)  # index within (t,r)
        # compute start positions per (t, r) via sorted order
        key = t * 4 + r
        starts = np.zeros(NT * 4, dtype=np.int64)
        cnt = np.bincount(key, minlength=NT * 4)
        starts[1:] = np.cumsum(cnt)[:-1]
        pos_in = np.arange(len(sp)) - starts[key]
        # column of each edge: group layout lookup
        colmap = np.zeros((NT, 4), dtype=np.int64)
        for g in groups:
            for ti in g["tiles"]:
                cols = g["tile_cols"][ti]
                # cols are ordered r-major with K[ti][r] entries each
                o = 0
                for r_ in range(4):
                    colmap[ti, r_] = cols[o] if K[ti][r_] > 0 else 0
                    o += K[ti][r_]
        ecol = colmap[t, r] + pos_in // 128
        erow = pos_in % 128

        idx16 = np.zeros((128, totch * 8), dtype=np.int16)
        rel = (sp - r * RANGE).astype(np.int16)
        for s in range(8):
            idx16[16 * s + erow % 16, ecol * 8 + erow // 16] = rel
        oh = np.zeros((128, totch * 128), dtype=ml_dtypes.bfloat16)
        oh[erow, ecol * 128 + w] = 1.0
        staged.append((idx16, oh))
    return staged, groups, totch


def _build(groups, totch):
    nc = bacc.Bacc("TRN2", target_bir_lowering=False, debug=False,
                   num_devices=N_CORES, num_swdge_queues=4)
    NGRP = len(groups)
    xT = nc.dram_tensor("xT", [F1, NPAD], BF, kind="ExternalInput")
    w1 = nc.dram_tensor("w1", [F1, H], BF, kind="ExternalInput")
    w2 = nc.dram_tensor("w2", [H, O], BF, kind="ExternalInput")
    a1s = nc.dram_tensor("a1s", [H], F32, kind="ExternalInput")
    a1d = nc.dram_tensor("a1d", [H], F32, kind="ExternalInput")
    b1 = nc.dram_tensor("b1", [H], F32, kind="ExternalInput")
    a2s = nc.dram_tensor("a2s", [O], F32, kind="ExternalInput")
    a2d = nc.dram_tensor("a2d", [O], F32, kind="ExternalInput")
    b2 = nc.dram_tensor("b2", [O], F32, kind="ExternalInput")
    idxs = nc.dram_tensor("idxs", [128, totch * 8], mybir.dt.int16,
                          kind="ExternalInput")
    ohd = nc.dram_tensor("ohd", [128, totch * 128], BF, kind="ExternalInput")
    identd = nc.dram_tensor("identd", [128, 128], BF, kind="ExternalInput")
    outp = nc.dram_tensor("outp", [NPAD, O], F32, kind="ExternalOutput")

    with tile.TileContext(nc) as tc:
        with tc.tile_pool(name="const", bufs=1) as cp, \
             tc.tile_pool(name="dram", bufs=1, space="DRAM") as dp, \
             tc.tile_pool(name="xp", bufs=3) as xp, \
             tc.tile_pool(name="ohp", bufs=3) as ohp, \
             tc.tile_pool(name="gp", bufs=3) as gp, \
             tc.tile_pool(name="ep", bufs=2) as ep, \
             tc.tile_pool(name="wp", bufs=3) as wp, \
             tc.tile_pool(name="ps", bufs=2, space="PSUM") as pp:

            nc.gpsimd.load_library(mlp)
            tc.no_sync_barrier()

            # ---- constants ----
            w1a = cp.tile([128, H], BF); nc.sync.dma_start(out=w1a[:], in_=w1[0:128, :])
            w1b = cp.tile([128, H], BF); nc.sync.dma_start(out=w1b[:], in_=w1[128:256, :])
            w2t = cp.tile([H, O], BF); nc.sync.dma_start(out=w2t[:], in_=w2[:])
            def brow(name, vec, n):
                tl = cp.tile([128, n], F32, tag=name)
                nc.sync.dma_start(out=tl[:], in_=vec[None, :].to_broadcast([128, n]))
                return tl
            a1s_r = brow("a1s", a1s, H); a1d_r = brow("a1d", a1d, H)
            b1_r = brow("b1", b1, H)
            a2s_r = brow("a2s", a2s, O); a2d_r = brow("a2d", a2d, O)
            b2_r = brow("b2", b2, O)
            identb = cp.tile([128, 128], BF)
            nc.sync.dma_start(out=identb[:], in_=identd[:])
            idxt = cp.tile([128, totch * 8], mybir.dt.int16)
            nc.sync.dma_start(out=idxt[:], in_=idxs[:])

            # ---- DRAM intermediates ----
            comb1_l = dp.tile([NPAD, EW], BF)
            comb1_f = dp.tile([NFULL, EW], BF)
            ad1_l = dp.tile([NPAD, 1], BF)
            comb2_l = dp.tile([NPAD, EW], BF)
            comb2_f = dp.tile([NFULL, EW], BF)
            ad2_l = dp.tile([NPAD, 1], BF)

            # ---- phase 1: h1 = x @ W1 (+ logits), packed bf16 rows ----
            for g in groups:
                tiles = g["tiles"]; gt = len(tiles); g0 = tiles[0]
                xs0 = xp.tile([128, gt * 128], BF, tag="xs0")
                nc.sync.dma_start(out=xs0[:], in_=xT[0:128, g0 * 128:(g0 + gt) * 128])
                xs1 = xp.tile([128, gt * 128], BF, tag="xs1")
                nc.sync.dma_start(out=xs1[:], in_=xT[128:256, g0 * 128:(g0 + gt) * 128])
                gcomb = xp.tile([128, gt * EW], BF, tag="gcomb")
                gcf32 = gcomb[:].bitcast(F32)
                adg = xp.tile([128, gt], F32, tag="adg")
                adgb = xp.tile([128, gt], BF, tag="adgb")
                nc.vector.memset(gcomb[:], 0.0)
                nc.vector.memset(
                    gcomb[:].rearrange("p (t k) -> p t k", k=EW)[:, :, H + 2:H + 3], 1.0)
                for i in range(gt):
                    hp = pp.tile([128, H], F32, tag="hp")
                    nc.tensor.matmul(out=hp[:], lhsT=xs0[:, i * 128:(i + 1) * 128],
                                     rhs=w1a[:], start=True, stop=False)
                    nc.tensor.matmul(out=hp[:], lhsT=xs1[:, i * 128:(i + 1) * 128],
                                     rhs=w1b[:], start=False, stop=True)
                    nc.scalar.activation(out=gcomb[:, i * EW:i * EW + H], in_=hp[:],
                                         func=AF.Copy)
                    scr = wp.tile([128, H], F32, tag="scr")
                    ascol = wp.tile([128, 1], F32, tag="ascol")
                    nc.vector.scalar_tensor_tensor(
                        out=scr[:], in0=hp[:], scalar=1.0, in1=a1s_r[:],
                        op0=ALU.mult, op1=ALU.mult, accum_out=ascol[:])
                    nc.vector.scalar_tensor_tensor(
                        out=scr[:], in0=hp[:], scalar=1.0, in1=a1d_r[:],
                        op0=ALU.mult, op1=ALU.mult, accum_out=adg[:, i:i + 1])
                    nc.vector.tensor_copy(out=gcf32[:, i * (EW // 2) + H // 2:
                                                    i * (EW // 2) + H // 2 + 1],
                                          in_=ascol[:])
                nc.vector.tensor_copy(out=adgb[:], in_=adg[:])
                nc.sync.dma_start(
                    out=comb1_l[:].rearrange("(t p) k -> p t k", p=128)
                    [:, g0:g0 + gt, :], in_=gcomb[:])
                nc.sync.dma_start(
                    out=ad1_l[:].rearrange("(t p) one -> p t one", p=128)
                    [:, g0:g0 + gt, :], in_=adgb[:])

            # ---- phase 2: all-gather layer-1 table ----
            nc.gpsimd.collective_compute(
                "AllGather", ALU.bypass, replica_groups=[list(range(N_CORES))],
                ins=[comb1_l[:].opt()], outs=[comb1_f[:].opt()])

            def edge_layer(comb_f, comb_l, ad_l, FW, last,
                           ans_r, and_r, bias_r, comb_out, ad_out):
                FWU = FW + 3        # h | as(2) | one
                qi = 0
                for g in groups:
                    tiles = g["tiles"]; gt = len(tiles); g0 = tiles[0]
                    ngc = g["n"]; c0 = g["start"]
                    ohg = ohp.tile([128, ngc * 128], BF, tag="ohg")
                    _oeng = nc.sync if (c0 // max(ngc, 1)) % 2 == 0 else nc.scalar
                    _oeng.dma_start(out=ohg[:],
                                    in_=ohd[:, c0 * 128:(c0 + ngc) * 128])
                    gbuf = gp.tile([128, ngc * 128], BF, tag="gbuf")
                    for (s_r, n_r), rbase in zip(g["spans"],
                                                 range(0, NFULL, RANGE)):
                        # HW limit: ~1024 idx per dma_gather (128B/partition
                        # of wrapped idx data); split spans into <=8-chunk ops
                        for p0 in range(0, n_r, 8):
                            pn = min(8, n_r - p0)
                            s_p = s_r + p0
                            gview = gbuf[:, (s_p - c0) * 128:(s_p - c0 + pn) * 128] \
                                .rearrange("p (c k) -> p c k", k=128)
                            nc.gpsimd.dma_gather(
                                gview, comb_f[rbase:rbase + RANGE, :],
                                idxt[:, s_p * 8:(s_p + pn) * 8],
                                pn * 128, pn * 128, EW, queue_num=qi % 4)
                            qi += 1
                    # self-loop rows: sequential load of local table rows
                    gself = gp.tile([128, gt * EW], BF, tag="gself")
                    nc.sync.dma_start(
                        out=gself[:],
                        in_=comb_l[:].rearrange("(t p) k -> p t k", p=128)
                        [:, g0:g0 + gt, :])
                    # ad tiles: [128, gt*128] broadcast (free axis) + diag col
                    adw = ep.tile([128, gt * 128], BF, tag="adw")
                    nc.sync.dma_start(
                        out=adw[:],
                        in_=ad_l[g0 * 128:(g0 + gt) * 128, 0:1]
                        .rearrange("n one -> one n").to_broadcast([128, gt * 128]))
                    adc = ep.tile([128, gt], BF, tag="adc")
                    nc.sync.dma_start(
                        out=adc[:],
                        in_=ad_l[:].rearrange("(t p) one -> p t one", p=128)
                        [:, g0:g0 + gt, :])
                    # ad per edge (gathered chunks): fused onehot*ad + rowsum
                    ade = ep.tile([128, ngc], F32, tag="ade")
                    for t_i, t in enumerate(tiles):
                        for c in g["tile_cols"][t]:
                            osel = wp.tile([128, 128], BF, tag="osel")
                            nc.vector.scalar_tensor_tensor(
                                out=osel[:], in0=ohg[:, (c - c0) * 128:(c - c0 + 1) * 128],
                                scalar=1.0, in1=adw[:, t_i * 128:(t_i + 1) * 128],
                                op0=ALU.mult, op1=ALU.mult,
                                accum_out=ade[:, c - c0:c - c0 + 1])
                    # scores -> ex  (gathered chunks)
                    gf32 = gbuf[:].bitcast(F32).rearrange(
                        "p (c k) -> p c k", k=EW // 2)
                    asv = gf32[:, :, FW // 2:FW // 2 + 1].squeeze(2)
                    et = ep.tile([128, ngc], F32, tag="et")
                    nc.vector.tensor_tensor(out=et[:], in0=asv, in1=ade[:], op=ALU.add)
                    lrt = ep.tile([128, ngc], F32, tag="lrt")
                    nc.vector.scalar_tensor_tensor(
                        out=lrt[:], in0=et[:], scalar=NEG_SLOPE, in1=et[:],
                        op0=ALU.mult, op1=ALU.max)
                    ext = ep.tile([128, ngc], F32, tag="ext")
                    nc.scalar.activation(out=ext[:], in_=lrt[:], func=AF.Exp)
                    # scores -> ex  (self chunks)
                    gsf32 = gself[:].bitcast(F32).rearrange(
                        "p (t k) -> p t k", k=EW // 2)
                    asv_s = gsf32[:, :, FW // 2:FW // 2 + 1].squeeze(2)
                    ets = ep.tile([128, gt], F32, tag="ets")
                    nc.vector.tensor_tensor(out=ets[:], in0=asv_s, in1=adc[:], op=ALU.add)
                    lrs = ep.tile([128, gt], F32, tag="lrs")
                    nc.vector.scalar_tensor_tensor(
                        out=lrs[:], in0=ets[:], scalar=NEG_SLOPE, in1=ets[:],
                        op0=ALU.mult, op1=ALU.max)
                    exs = ep.tile([128, gt], F32, tag="exs")
                    nc.scalar.activation(out=exs[:], in_=lrs[:], func=AF.Exp)

                    # per-tile: weight + scatter-accumulate + epilogue
                    if last:
                        gout = ep.tile([128, gt * O], F32, tag="gout")
                    else:
                        gc2 = ep.tile([128, gt * EW], BF, tag="gc2")
                        gc2f = gc2[:].bitcast(F32)
                        adg2 = ep.tile([128, gt], F32, tag="adg2")
                        adg2b = ep.tile([128, gt], BF, tag="adg2b")
                        nc.vector.memset(gc2[:], 0.0)
                        nc.vector.memset(
                            gc2[:].rearrange("p (t k) -> p t k", k=EW)
                            [:, :, O + 2:O + 3], 1.0)
                    for t_i, t in enumerate(tiles):
                        ps = pp.tile([128, FWU], F32, tag="pe")
                        wts = wp.tile([128, FWU], BF, tag="wts")
                        nc.scalar.activation(
                            out=wts[:], in_=gself[:, t_i * EW:t_i * EW + FWU],
                            func=AF.Copy, scale=exs[:, t_i:t_i + 1])
                        nc.tensor.matmul(out=ps[:], lhsT=identb[:], rhs=wts[:],
                                         start=True, stop=False)
                        cols = g["tile_cols"][t]
                        for j, c in enumerate(cols):
                            wt = wp.tile([128, FWU], BF, tag="wt")
                            nc.scalar.activation(
                                out=wt[:], in_=gbuf[:, (c - c0) * 128:(c - c0) * 128 + FWU],
                                func=AF.Copy, scale=ext[:, c - c0:c - c0 + 1])
                            nc.tensor.matmul(
                                out=ps[:], lhsT=ohg[:, (c - c0) * 128:(c - c0 + 1) * 128],
                                rhs=wt[:], start=False, stop=(j == len(cols) - 1))
                        # epilogue
                        rec = wp.tile([128, 1], F32, tag="rec")
                        nc.vector.reciprocal(out=rec[:], in_=ps[:, FWU - 1:FWU])
                        if last:
                            nc.vector.scalar_tensor_tensor(
                                out=gout[:, t_i * O:(t_i + 1) * O], in0=ps[:, 0:FW],
                                scalar=rec[:], in1=bias_r[:],
                                op0=ALU.mult, op1=ALU.add)
                            continue
                        o1 = wp.tile([128, FW], F32, tag="o1")
                        nc.vector.scalar_tensor_tensor(
                            out=o1[:], in0=ps[:, 0:FW], scalar=rec[:],
                            in1=bias_r[:], op0=ALU.mult, op1=ALU.add)
                        o1b = wp.tile([128, FW], BF, tag="o1b")
                        nc.scalar.activation(out=o1b[:], in_=o1[:], func=AF.Relu)
                        trp = pp.tile([FW, 128], BF, tag="tr")
                        nc.tensor.transpose(out=trp[:], in_=o1b[:], identity=identb[:])
                        o1T = wp.tile([FW, 128], BF, tag="o1T")
                        nc.vector.tensor_copy(out=o1T[:], in_=trp[:])
                        h2p = pp.tile([128, O], F32, tag="h2")
                        nc.tensor.matmul(out=h2p[:], lhsT=o1T[:], rhs=w2t[:],
                                         start=True, stop=True)
                        nc.scalar.activation(out=gc2[:, t_i * EW:t_i * EW + O],
                                             in_=h2p[:], func=AF.Copy)
                        scr2 = wp.tile([128, O], F32, tag="scr2")
                        as2 = wp.tile([128, 1], F32, tag="as2")
                        nc.vector.scalar_tensor_tensor(
                            out=scr2[:], in0=h2p[:], scalar=1.0, in1=ans_r[:],
                            op0=ALU.mult, op1=ALU.mult, accum_out=as2[:])
                        nc.vector.scalar_tensor_tensor(
                            out=scr2[:], in0=h2p[:], scalar=1.0, in1=and_r[:],
                            op0=ALU.mult, op1=ALU.mult,
                            accum_out=adg2[:, t_i:t_i + 1])
                        nc.vector.tensor_copy(out=adg2b[:, t_i:t_i + 1],
                                              in_=adg2[:, t_i:t_i + 1])
                        nc.vector.tensor_copy(
                            out=gc2f[:, t_i * (EW // 2) + O // 2:
                                     t_i * (EW // 2) + O // 2 + 1], in_=as2[:])
                    if last:
                        nc.sync.dma_start(
                            out=outp[:].rearrange("(t p) k -> p t k", p=128)
                            [:, g0:g0 + gt, :], in_=gout[:])
                    else:
                        nc.sync.dma_start(
                            out=comb_out[:].rearrange("(t p) k -> p t k", p=128)
                            [:, g0:g0 + gt, :], in_=gc2[:])
                        nc.sync.dma_start(
                            out=ad_out[:].rearrange("(t p) one -> p t one", p=128)
                            [:, g0:g0 + gt, :], in_=adg2b[:])

            # ---- phase 3: edge layer 1 (fused layer-2 GEMM) ----
            edge_layer(comb1_f, comb1_l, ad1_l, H, False,
                       a2s_r, a2d_r, b1_r, comb2_l, ad2_l)

            # ---- phase 4: all-gather layer-2 table ----
            nc.gpsimd.collective_compute(
                "AllGather", ALU.bypass, replica_groups=[list(range(N_CORES))],
                ins=[comb2_l[:].opt()], outs=[comb2_f[:].opt()])

            # ---- phase 5: edge layer 2 ----
            edge_layer(comb2_f, comb2_l, ad2_l, O, True,
                       None, None, b2_r, None, None)

    nc.compile()
    return nc


def kernel(x, edge_index, W1, a1_src, a1_dst, b1, W2, a2_src, a2_dst, b2):
    x = np.asarray(x, dtype=np.float32)
    staged, groups, totch = _prep_edges(np.asarray(edge_index))
    nc = _build(groups, totch)

    ident = np.eye(128, dtype=ml_dtypes.bfloat16)
    common = dict(
        w1=np.asarray(W1, np.float32).astype(ml_dtypes.bfloat16),
        w2=np.asarray(W2, np.float32).astype(ml_dtypes.bfloat16),
        a1s=np.asarray(a1_src, np.float32), a1d=np.asarray(a1_dst, np.float32),
        b1=np.asarray(b1, np.float32), a2s=np.asarray(a2_src, np.float32),
        a2d=np.asarray(a2_dst, np.float32), b2=np.asarray(b2, np.float32),
        identd=ident,
    )
    in_maps = []
    for c in range(N_CORES):
        idx16, oh = staged[c]
        xT = np.zeros((F1, NPAD), ml_dtypes.bfloat16)
        xT[:, :NPC] = x[c * NPC:(c + 1) * NPC].T.astype(ml_dtypes.bfloat16)
        in_maps.append(dict(common, xT=xT, idxs=idx16, ohd=oh))

    global _LAST_NC, _LAST_INMAPS
    _LAST_NC, _LAST_INMAPS = nc, in_maps
    res = run_bass_kernel_spmd(nc, in_maps, core_ids=list(range(N_CORES)))
    out = np.concatenate(
        [res.results[c]["outp"][:NPC] for c in range(N_CORES)], axis=0)
    return out.astype(np.float32)
